# revision 29
# baseline (speedup 1.0000x reference)
"""Causal single-head attention on 8 Trainium2 NeuronCores — collective-free.

Problem: x [4, 2048, 1024] f32; Wq/Wk/Wv [1024, 1024] f32.
  q,k,v = x@W*; out = softmax(causal(q k^T / sqrt(d))) @ v.

Since q and k are never needed individually, the host folds
M = Wq @ Wk^T (fp32, free) and the device computes
  scores = (x @ M) @ x^T        (u-proj + scores; k-projection gone)
  out    = (att @ x) @ Wv       (zT + out-proj; v-projection moved
                                 after attention, now purely local)
so NO collectives are needed: every core only touches its own queries.

Sharding: 8 cores = 4 batches x 2 query-halves. Causal load balancing:
every 128-query block is split between the pair (64 rows each), so both
cores see IDENTICAL causal depth profiles and one program serves all
cores (SPMD). Queries group into two 512-query supers (super 0 = blocks
8..15, super 1 = blocks 0..7, ascending depth); per-core causal
structure lives in an additive -57344 mask (host-built, fp8e5).

Scores and zT are N-trimmed at 64-query granularity (exact causal:
columns below VSTART(s, kt) are never computed; E is pre-zeroed so the
denominator can read whole 128-column slots). The softmax denominator
uses E-as-stationary N=2 matmuls; normalization happens on the host.

All input DMAs ride ONE queue (sync) in need order — a second concurrent
input queue steals bandwidth from the critical path (measured: the
critical queue crawled at ~60 GB/s while the other streamed). Host
layouts keep per-partition rows large and contiguous. Outputs go on the
gpsimd queue.

All matmul operands are fp16; accumulation stays fp32 in PSUM. A dummy
warm-up matmul burst at t~0 lifts the PE HAM clock gate (1.2 -> 2.4 GHz)
while the first input DMAs land (~8 us until first bytes arrive).
"""

import os
import sys
from contextlib import ExitStack

sys.path.insert(0, "/opt/trn_rl_repo")

import numpy as np

import concourse.bass as bass  # noqa: F401
import concourse.tile as tile
from concourse import bacc, mybir
from concourse.bass_utils import run_bass_kernel_spmd

B, T, D = 4, 2048, 1024
P = 128                 # partitions
DC = D // P             # 8 feature chunks
TCH = T // P            # 16 token chunks
KC = 4                  # xt key-chunk DMA granularity (512 keys each)
QSUP = 512              # queries per super
NSUP = 2                # supers per core
NQ = QSUP * NSUP        # 1024 queries per core
SLOT_KT = (16, 8)       # score key tiles per super slot (compile-time)
NKT = sum(SLOT_KT)      # 24
SCALE = 1.0 / 32.0      # 1/sqrt(D)
MASK_NEG = -57344.0     # representable in fp8e5; exp((s+m)/32) == 0
NWARM = 20              # PE warm-up burst length

# per-(super, qs-slot) key-tile counts for the denominator (depth of the
# deeper 64-query half of each slot; identical on every core)
AVKT = ((10, 12, 14, 16), (2, 4, 6, 8))
# key-tile depth of each 64-query group (8 per super, ascending)
DEPTHS = (tuple(range(9, 17)), tuple(range(1, 9)))


# scores/zT N-trim: first valid query column for (super, kt), 64-granular
def _vstart(s, kt):
    return 64 * sum(1 for dep in DEPTHS[s] if dep <= kt)


F16 = mybir.dt.float16
F32 = mybir.dt.float32
F8E5 = mybir.dt.float8e5
F8E4 = mybir.dt.float8e4

# scores in fp8e4 with DoubleRow (2x PE rate). Measured rel err ~1.4e-2
# vs the 2e-2 gate (fp16 path: 4.7e-4). Set False to fall back to fp16.
SCORES_FP8 = True
SDT = F8E4 if SCORES_FP8 else F16

_CACHE = {}

last_exec_time_ns = None  # set when BASS_KERNEL_TRACE=1


def _build_program():
    nc = bacc.Bacc("TRN2", target_bir_lowering=False, debug=False, num_devices=8)

    # DMA is descriptor-rate limited (~4.3us per 128-partition transfer,
    # size-independent below ~2MB): pack [mw-half | xq-super] pairs so the
    # critical first bytes ride ONE descriptor set each.
    uin_d = [nc.dram_tensor(f"uin{i}", [P, 8192], F16, kind="ExternalInput")
             for i in range(2)]
    msk_d = nc.dram_tensor("msk", [P, NKT, QSUP], F8E5, kind="ExternalInput")
    xt_d = [nc.dram_tensor(f"xt{c}", [P, DC, T // 2], SDT,
                           kind="ExternalInput") for c in range(2)]
    xtok_d = nc.dram_tensor("xtok", [P, TCH, D], F16, kind="ExternalInput")
    wv_d = nc.dram_tensor("wv", [P, DC, D], F16, kind="ExternalInput")
    # [s, p, qs, e]: 8KB contiguous per partition -> fast DMA rows
    out_d = nc.dram_tensor("out", [NSUP, P, 4, D], F16, kind="ExternalOutput")
    dsum_d = nc.dram_tensor("dsum", [P, 8], F32, kind="ExternalOutput")

    with tile.TileContext(nc) as tc, ExitStack() as stack:
        p_mw = stack.enter_context(tc.tile_pool(name="mw", bufs=1))
        p_xq = stack.enter_context(tc.tile_pool(name="xq", bufs=1))
        p_xt = stack.enter_context(tc.tile_pool(name="xt", bufs=1))
        p_xtok = stack.enter_context(tc.tile_pool(name="xtok", bufs=1))
        p_wv = stack.enter_context(tc.tile_pool(name="wv", bufs=1))
        p_ut = stack.enter_context(tc.tile_pool(name="ut", bufs=1))
        p_e = stack.enter_context(tc.tile_pool(name="e", bufs=1))
        p_zt = stack.enter_context(tc.tile_pool(name="ztt", bufs=1))
        p_sm = stack.enter_context(tc.tile_pool(name="sm", bufs=2))
        p_out = stack.enter_context(tc.tile_pool(name="outp", bufs=1))
        p_misc = stack.enter_context(tc.tile_pool(name="misc", bufs=1))
        ps512 = stack.enter_context(tc.tile_pool(name="ps512", bufs=3, space="PSUM"))
        pszt = stack.enter_context(tc.tile_pool(name="pszt", bufs=2, space="PSUM"))

        # ---- constants + warmup ----
        ones_t = p_misc.tile([P, QSUP], F16, tag="ones")
        nc.gpsimd.memset(ones_t[:], 1.0)
        for w in range(NWARM):
            acc = ps512.tile([P, QSUP], F32, tag="ps512")
            nc.tensor.matmul(acc[:], ones_t[:, 0:P], ones_t[:],
                             start=True, stop=True)

        # ---- input loads: ONE queue, strict need order ----
        uin_t = [p_mw.tile([P, 8192], F16, tag=f"uin{i}", name=f"uin{i}")
                 for i in range(2)]
        nc.sync.dma_start(uin_t[0][:], uin_d[0].ap())
        nc.sync.dma_start(uin_t[1][:], uin_d[1].ap())
        m_all = p_misc.tile([P, NKT, QSUP], F8E5, tag="msk")
        nc.sync.dma_start(m_all[:], msk_d.ap())
        xt_t = []
        for c in range(2):
            xt = p_xt.tile([P, DC, T // 2], SDT, tag=f"xt{c}", name=f"xt{c}")
            nc.sync.dma_start(xt[:], xt_d[c].ap())
            xt_t.append(xt)
        xtok_t = p_xtok.tile([P, TCH, D], F16, tag="xtok")
        for tk in range(0, TCH, 8):
            nc.sync.dma_start(xtok_t[:, tk:tk + 8, :],
                              xtok_d.ap()[:, tk:tk + 8, :])
        wv_t = p_wv.tile([P, DC, D], F16, tag="wv")
        nc.sync.dma_start(wv_t[:], wv_d.ap())

        # ---- u-projection: ut[s] = (x @ M)^T for own queries ----
        # uin[i] per-partition layout: [mw ecs (4i..4i+3) x dc x e | xq_i]
        def mw_ap(ec, dc):
            u = uin_t[ec // 4]
            off = (ec % 4) * 1024 + dc * P
            return u[:, off:off + P]

        def xq_ap(s, dc):
            return uin_t[s][:, 4096 + dc * QSUP:4096 + (dc + 1) * QSUP]

        ut_s = []
        for s in range(NSUP):
            ut = p_ut.tile([P, DC, QSUP], SDT, tag=f"ut{s}", name=f"ut{s}")
            ut_s.append(ut)
            for ec in range(DC):
                acc = ps512.tile([P, QSUP], F32, tag="ps512")
                for dc in range(DC):
                    nc.tensor.matmul(
                        acc[:], mw_ap(ec, dc), xq_ap(s, dc),
                        start=(dc == 0), stop=(dc == DC - 1))
                nc.scalar.copy(ut[:, ec, :], acc[:])

        # ---- scores -> exp -> E ----
        e_ts = [
            p_e.tile([P, SLOT_KT[0], QSUP], F16, tag="e0", name="e0"),
            p_e.tile([P, SLOT_KT[1], QSUP], F16, tag="e1", name="e1"),
        ]
        # pre-zero E (during the DMA wait; engines are idle): the 64-trim
        # leaves sub-vstart columns unwritten, but the denominator reads
        # whole 128-column slots — zeros keep it exact.
        nc.vector.memset(e_ts[0][:], 0.0)
        nc.gpsimd.memset(e_ts[1][:], 0.0)
        KT_BASE = (0, SLOT_KT[0])
        score_order = ([(0, kt) for kt in range(SLOT_KT[0])]
                       + [(1, kt) for kt in range(SLOT_KT[1])])
        for s, kt in score_order:
            vs = _vstart(s, kt)
            acc = ps512.tile([P, QSUP], F32, tag="ps512")
            xt = xt_t[kt // 8]
            kcol = (kt % 8) * P
            if SCORES_FP8:
                # DoubleRow: 2 contraction chunks per instruction, 2x rate
                for ep in range(DC // 2):
                    nc.tensor.matmul(
                        acc[:, vs:], xt[:, 2 * ep:2 * ep + 2, kcol:kcol + P],
                        ut_s[s][:, 2 * ep:2 * ep + 2, vs:],
                        start=(ep == 0), stop=(ep == DC // 2 - 1),
                        perf_mode=mybir.MatmulPerfMode.DoubleRow)
            else:
                for ec in range(DC):
                    nc.tensor.matmul(
                        acc[:, vs:], xt[:, ec, kcol:kcol + P],
                        ut_s[s][:, ec, vs:],
                        start=(ec == 0), stop=(ec == DC - 1))
            sm_t = p_sm.tile([P, QSUP], F16, tag="sm")
            nc.vector.tensor_add(sm_t[:, vs:], acc[:, vs:],
                                 m_all[:, KT_BASE[s] + kt, vs:])
            nc.scalar.activation(e_ts[s][:, kt, vs:], sm_t[:, vs:],
                                 mybir.ActivationFunctionType.Exp,
                                 scale=SCALE)

        # ---- denominator: d = sum_k E via N=2 matmuls ----
        d_all = p_misc.tile([P, 8], F32, tag="dall")
        for s in range(NSUP):
            e_t = e_ts[s]
            for qs in range(4):
                nav = AVKT[s][qs]
                d_acc = ps512.tile([P, 2], F32, tag="ps512", name="d_acc")
                for kt in range(nav):
                    nc.tensor.matmul(d_acc[:],
                                     e_t[:, kt, qs * P:(qs + 1) * P],
                                     ones_t[:, 0:2],
                                     start=(kt == 0), stop=(kt == nav - 1))
                g = s * 4 + qs
                nc.vector.tensor_copy(d_all[:, g:g + 1], d_acc[:, 0:1])
        nc.sync.dma_start(dsum_d.ap(), d_all[:])

        # ---- zT = x_tok^T @ E per super, then out = zT^T @ Wv ----
        # zT groups are per (super, dc-pair): [P, 2, 512] PSUM (one bank
        # per dc), 512-wide vs-trimmed moving E — same cycles as the AV
        # matmul but the fewest Tensor instructions (the 120KB Tensor
        # instruction stream is demand-paged in 16KB DMAs; its last page
        # arriving late gates the teardown).
        # zT(s1) is emitted before out-proj(s0) so its SBUF copies hide
        # under out-proj(s0)'s matmuls instead of stalling the tail.
        ztt_s = []
        for s in range(NSUP):
            e_t = e_ts[s]
            # uin tiles are dead after u-proj: reuse their SBUF (WAR by tag)
            ztt = p_mw.tile([P, DC, QSUP], F16, tag=f"uin{s}", name=f"ztt{s}")
            ztt_s.append(ztt)
            for dp in range(4):
                zt = pszt.tile([P, 2, QSUP], F32, tag="zt")
                for kt in range(SLOT_KT[s]):
                    vs = _vstart(s, kt)
                    for dc in range(2):
                        dcg = dp * 2 + dc
                        nc.tensor.matmul(
                            zt[:, dc, vs:],
                            xtok_t[:, kt, dcg * P:(dcg + 1) * P],
                            e_t[:, kt, vs:],
                            start=(kt == 0), stop=(kt == SLOT_KT[s] - 1))
                nc.vector.tensor_copy(
                    ztt[:, dp * 2:(dp + 1) * 2, :], zt[:])
        for s in range(NSUP):
            for qp in range(2):
                o_t = p_out.tile([P, 2, D], F16, tag=f"o{s}{qp}",
                                 name=f"o{s}{qp}")
                for qh in range(2):
                    qs = qp * 2 + qh
                    for eh in range(2):
                        acc = ps512.tile([P, QSUP], F32, tag="ps512")
                        for dcg in range(DC):
                            nc.tensor.matmul(
                                acc[:], ztt_s[s][:, dcg, qs * P:(qs + 1) * P],
                                wv_t[:, dcg, eh * QSUP:(eh + 1) * QSUP],
                                start=(dcg == 0), stop=(dcg == DC - 1))
                        nc.scalar.copy(
                            o_t[:, qh, eh * QSUP:(eh + 1) * QSUP], acc[:])
                if s == 1 and qp == 1:
                    # final store: split by partition halves across two
                    # queues to halve the ~4.3us descriptor-service floor
                    nc.sync.dma_start(
                        out_d.ap()[s][0:64, qp * 2:qp * 2 + 2], o_t[0:64])
                    nc.gpsimd.dma_start(
                        out_d.ap()[s][64:P, qp * 2:qp * 2 + 2], o_t[64:P])
                else:
                    nc.sync.dma_start(
                        out_d.ap()[s][:, qp * 2:qp * 2 + 2], o_t[:])

    nc.compile()
    return nc


def _prep_weights(Wq32, Wk32, Wv16):
    """Host-side weight prep (shared by all cores)."""
    M16 = (Wq32 @ Wk32.T).astype(np.float16)
    mw = np.ascontiguousarray(
        M16.reshape(DC, P, DC, P).transpose(1, 2, 0, 3))    # [p, ec, dc, e]
    mw = mw.reshape(P, 2, 4096)                              # per mw-half
    wv = np.ascontiguousarray(Wv16.reshape(DC, P, D).swapaxes(0, 1))
    return mw, wv


def _core_tq(h):
    """Queries for core parity h: every 128-block is split between the
    pair (rows [h*64, h*64+64)), so both cores see identical causal depth
    profiles. Super 0 = blocks 8..15 (ascending depth), super 1 = 0..7."""
    return np.concatenate(
        [bl * P + h * 64 + np.arange(64) for bl in range(8, 16)]
        + [bl * P + h * 64 + np.arange(64) for bl in range(0, 8)])


def _prep_core_inputs(xT16, xtok16, mw, wv, b, h):
    """Host-side shard prep for core (batch b, half h)."""
    tq = _core_tq(h)

    xTb = xT16[b]                                          # [D, T] fp16
    xq = xTb[:, tq].reshape(DC, P, NSUP, QSUP).transpose(2, 1, 0, 3)
    xq = xq.reshape(NSUP, P, 4096)
    uin = [np.ascontiguousarray(
        np.concatenate([mw[:, i], xq[i]], axis=1)) for i in range(2)]
    xt = np.ascontiguousarray(
        xTb.reshape(DC, P, 2, T // 2).transpose(2, 1, 0, 3))
    if SCORES_FP8:
        import ml_dtypes
        xt = xt.astype(ml_dtypes.float8_e4m3)

    masks = np.empty((NKT, P, QSUP), dtype=np.float16)
    base = 0
    for s in range(NSUP):
        kidx = np.arange(SLOT_KT[s] * P).reshape(SLOT_KT[s], P, 1)
        tqs = tq[s * QSUP:(s + 1) * QSUP].reshape(1, 1, QSUP)
        masks[base:base + SLOT_KT[s]] = np.where(
            kidx <= tqs, 0.0, MASK_NEG).astype(np.float16)
        base += SLOT_KT[s]
    import ml_dtypes
    masks = np.ascontiguousarray(masks.transpose(1, 0, 2)).astype(
        ml_dtypes.float8_e5m2)                              # [P, NKT, QSUP]

    im = {"uin0": uin[0], "uin1": uin[1], "wv": wv,
          "msk": masks, "xtok": xtok16[b]}
    for c in range(2):
        im[f"xt{c}"] = xt[c]
    return im, tq


def kernel(x, Wq, Wk, Wv):
    global last_exec_time_ns
    x = np.asarray(x, dtype=np.float32)
    assert x.shape == (B, T, D)

    if "nc" not in _CACHE:
        _CACHE["nc"] = _build_program()
    nc = _CACHE["nc"]

    x16 = x.astype(np.float16)
    xT16 = np.ascontiguousarray(x16.transpose(0, 2, 1))    # [B, D, T]
    xtok16 = np.ascontiguousarray(
        x16.reshape(B, TCH, P, D).transpose(0, 2, 1, 3))   # [B, P, TCH, D]
    mw, wv = _prep_weights(
        np.asarray(Wq, dtype=np.float32),
        np.asarray(Wk, dtype=np.float32),
        np.asarray(Wv, dtype=np.float16))

    in_maps = []
    row_maps = []
    for c in range(8):
        im, tq = _prep_core_inputs(xT16, xtok16, mw, wv, c // 2, c % 2)
        in_maps.append(im)
        row_maps.append(tq)

    trace = bool(os.environ.get("BASS_KERNEL_TRACE"))
    kw = {}
    if trace:
        kw = {"trace": True, "tmpdir": os.environ.get(
            "BASS_KERNEL_TRACE_DIR", "/tmp/kernel_trace")}
    res = run_bass_kernel_spmd(nc, in_maps, core_ids=list(range(8)), **kw)
    if trace:
        last_exec_time_ns = res.exec_time_ns

    out = np.empty((B, T, D), dtype=np.float32)
    for c in range(8):
        o = np.asarray(res.results[c]["out"], dtype=np.float32)  # [2, P, 4, D]
        o = o.transpose(0, 2, 1, 3).reshape(NQ, D)
        d = np.asarray(res.results[c]["dsum"], dtype=np.float32)  # [P, 8]
        o /= np.ascontiguousarray(d.T).reshape(NQ, 1)
        out[c // 2, row_maps[c]] = o
    return out


# revision 30
# speedup vs baseline: 1.0629x; 1.0629x over previous
"""Causal single-head attention on 8 Trainium2 NeuronCores — collective-free.

Problem: x [4, 2048, 1024] f32; Wq/Wk/Wv [1024, 1024] f32.
  q,k,v = x@W*; out = softmax(causal(q k^T / sqrt(d))) @ v.

Since q and k are never needed individually, the host folds
M = Wq @ Wk^T (fp32, free) and the device computes
  scores = (x @ M) @ x^T        (u-proj + scores; k-projection gone)
  out    = (att @ x) @ Wv       (zT + out-proj; v-projection moved
                                 after attention, now purely local)
so NO collectives are needed: every core only touches its own queries.

Sharding: 8 cores = 4 batches x 2 query-halves. Causal load balancing:
every 128-query block is split between the pair (64 rows each), so both
cores see IDENTICAL causal depth profiles and one program serves all
cores (SPMD). Queries group into two 512-query supers (super 0 = blocks
8..15, super 1 = blocks 0..7, ascending depth); per-core causal
structure lives in an additive -57344 mask (host-built, fp8e5).

Scores and zT are N-trimmed at 64-query granularity (exact causal:
columns below VSTART(s, kt) are never computed; E is pre-zeroed so the
denominator can read whole 128-column slots). The softmax denominator
uses E-as-stationary N=2 matmuls; normalization happens on the host.

All input DMAs ride ONE queue (sync) in need order — a second concurrent
input queue steals bandwidth from the critical path (measured: the
critical queue crawled at ~60 GB/s while the other streamed). Host
layouts keep per-partition rows large and contiguous. Outputs go on the
gpsimd queue.

All matmul operands are fp16; accumulation stays fp32 in PSUM. A dummy
warm-up matmul burst at t~0 lifts the PE HAM clock gate (1.2 -> 2.4 GHz)
while the first input DMAs land (~8 us until first bytes arrive).
"""

import os
import sys
from contextlib import ExitStack

sys.path.insert(0, "/opt/trn_rl_repo")

import numpy as np

import concourse.bass as bass  # noqa: F401
import concourse.tile as tile
from concourse import bacc, mybir
from concourse.bass_utils import run_bass_kernel_spmd

B, T, D = 4, 2048, 1024
P = 128                 # partitions
DC = D // P             # 8 feature chunks
TCH = T // P            # 16 token chunks
KC = 4                  # xt key-chunk DMA granularity (512 keys each)
QSUP = 512              # queries per super
NSUP = 2                # supers per core
NQ = QSUP * NSUP        # 1024 queries per core
SLOT_KT = (16, 8)       # score key tiles per super slot (compile-time)
NKT = sum(SLOT_KT)      # 24
SCALE = 1.0 / 32.0      # 1/sqrt(D)
MASK_NEG = -57344.0     # representable in fp8e5; exp((s+m)/32) == 0
NWARM = 20              # PE warm-up burst length

# per-(super, qs-slot) key-tile counts for the denominator (depth of the
# deeper 64-query half of each slot; identical on every core)
AVKT = ((10, 12, 14, 16), (2, 4, 6, 8))
# key-tile depth of each 64-query group (8 per super, ascending)
DEPTHS = (tuple(range(9, 17)), tuple(range(1, 9)))


# scores/zT N-trim: first valid query column for (super, kt), 64-granular
def _vstart(s, kt):
    return 64 * sum(1 for dep in DEPTHS[s] if dep <= kt)


F16 = mybir.dt.float16
F32 = mybir.dt.float32
F8E5 = mybir.dt.float8e5
F8E4 = mybir.dt.float8e4

# scores in fp8e4 with DoubleRow (2x PE rate). Measured: rel err 1.18e-2
# (passes the 2e-2 gate) BUT the extra power density trips a global ~88%
# utilization throttle that slows EVERY phase ~21% — net loss (148us vs
# 139us). Keep False.
SCORES_FP8 = False
SDT = F8E4 if SCORES_FP8 else F16

_CACHE = {}

last_exec_time_ns = None  # set when BASS_KERNEL_TRACE=1


def _build_program():
    nc = bacc.Bacc("TRN2", target_bir_lowering=False, debug=False, num_devices=8)

    # DMA is descriptor-rate limited (~4.3us per 128-partition transfer,
    # size-independent below ~2MB): pack [mw-half | xq-super] pairs so the
    # critical first bytes ride ONE descriptor set each.
    uin_d = [nc.dram_tensor(f"uin{i}", [P, 8192], F16, kind="ExternalInput")
             for i in range(2)]
    msk_d = nc.dram_tensor("msk", [P, NKT, QSUP], F8E5, kind="ExternalInput")
    xt_d = [nc.dram_tensor(f"xt{c}", [P, DC, T // 2], SDT,
                           kind="ExternalInput") for c in range(2)]
    xtok_d = nc.dram_tensor("xtok", [P, TCH, D], F16, kind="ExternalInput")
    wv_d = nc.dram_tensor("wv", [P, DC, D], F16, kind="ExternalInput")
    # [s, p, qs, e]: 8KB contiguous per partition -> fast DMA rows
    out_d = nc.dram_tensor("out", [NSUP, P, 4, D], F16, kind="ExternalOutput")
    dsum_d = nc.dram_tensor("dsum", [P, 8], F32, kind="ExternalOutput")

    with tile.TileContext(nc) as tc, ExitStack() as stack:
        p_mw = stack.enter_context(tc.tile_pool(name="mw", bufs=1))
        p_xq = stack.enter_context(tc.tile_pool(name="xq", bufs=1))
        p_xt = stack.enter_context(tc.tile_pool(name="xt", bufs=1))
        p_xtok = stack.enter_context(tc.tile_pool(name="xtok", bufs=1))
        p_wv = stack.enter_context(tc.tile_pool(name="wv", bufs=1))
        p_ut = stack.enter_context(tc.tile_pool(name="ut", bufs=1))
        p_e = stack.enter_context(tc.tile_pool(name="e", bufs=1))
        p_zt = stack.enter_context(tc.tile_pool(name="ztt", bufs=1))
        p_sm = stack.enter_context(tc.tile_pool(name="sm", bufs=2))
        p_out = stack.enter_context(tc.tile_pool(name="outp", bufs=1))
        p_misc = stack.enter_context(tc.tile_pool(name="misc", bufs=1))
        ps512 = stack.enter_context(tc.tile_pool(name="ps512", bufs=3, space="PSUM"))
        pszt = stack.enter_context(tc.tile_pool(name="pszt", bufs=2, space="PSUM"))

        # ---- constants + warmup ----
        ones_t = p_misc.tile([P, QSUP], F16, tag="ones")
        nc.gpsimd.memset(ones_t[:], 1.0)
        for w in range(NWARM):
            acc = ps512.tile([P, QSUP], F32, tag="ps512")
            nc.tensor.matmul(acc[:], ones_t[:, 0:P], ones_t[:],
                             start=True, stop=True)

        # ---- input loads: ONE queue, strict need order ----
        uin_t = [p_mw.tile([P, 8192], F16, tag=f"uin{i}", name=f"uin{i}")
                 for i in range(2)]
        nc.sync.dma_start(uin_t[0][:], uin_d[0].ap())
        nc.sync.dma_start(uin_t[1][:], uin_d[1].ap())
        m_all = p_misc.tile([P, NKT, QSUP], F8E5, tag="msk")
        nc.sync.dma_start(m_all[:], msk_d.ap())
        xt_t = []
        for c in range(2):
            xt = p_xt.tile([P, DC, T // 2], SDT, tag=f"xt{c}", name=f"xt{c}")
            nc.sync.dma_start(xt[:], xt_d[c].ap())
            xt_t.append(xt)
        xtok_t = p_xtok.tile([P, TCH, D], F16, tag="xtok")
        for tk in range(0, TCH, 8):
            nc.sync.dma_start(xtok_t[:, tk:tk + 8, :],
                              xtok_d.ap()[:, tk:tk + 8, :])
        wv_t = p_wv.tile([P, DC, D], F16, tag="wv")
        nc.sync.dma_start(wv_t[:], wv_d.ap())

        # ---- u-projection: ut[s] = (x @ M)^T for own queries ----
        # uin[i] per-partition layout: [mw ecs (4i..4i+3) x dc x e | xq_i]
        def mw_ap(ec, dc):
            u = uin_t[ec // 4]
            off = (ec % 4) * 1024 + dc * P
            return u[:, off:off + P]

        def xq_ap(s, dc):
            return uin_t[s][:, 4096 + dc * QSUP:4096 + (dc + 1) * QSUP]

        ut_s = []
        for s in range(NSUP):
            ut = p_ut.tile([P, DC, QSUP], SDT, tag=f"ut{s}", name=f"ut{s}")
            ut_s.append(ut)
            for ec in range(DC):
                acc = ps512.tile([P, QSUP], F32, tag="ps512")
                for dc in range(DC):
                    nc.tensor.matmul(
                        acc[:], mw_ap(ec, dc), xq_ap(s, dc),
                        start=(dc == 0), stop=(dc == DC - 1))
                nc.scalar.copy(ut[:, ec, :], acc[:])

        # ---- scores -> exp -> E ----
        e_ts = [
            p_e.tile([P, SLOT_KT[0], QSUP], F16, tag="e0", name="e0"),
            p_e.tile([P, SLOT_KT[1], QSUP], F16, tag="e1", name="e1"),
        ]
        # pre-zero E (during the DMA wait; engines are idle): the 64-trim
        # leaves sub-vstart columns unwritten, but the denominator reads
        # whole 128-column slots — zeros keep it exact.
        nc.vector.memset(e_ts[0][:], 0.0)
        nc.gpsimd.memset(e_ts[1][:], 0.0)
        KT_BASE = (0, SLOT_KT[0])
        score_order = ([(0, kt) for kt in range(SLOT_KT[0])]
                       + [(1, kt) for kt in range(SLOT_KT[1])])
        for s, kt in score_order:
            vs = _vstart(s, kt)
            acc = ps512.tile([P, QSUP], F32, tag="ps512")
            xt = xt_t[kt // 8]
            kcol = (kt % 8) * P
            if SCORES_FP8:
                # DoubleRow: 2 contraction chunks per instruction, 2x rate
                for ep in range(DC // 2):
                    nc.tensor.matmul(
                        acc[:, vs:], xt[:, 2 * ep:2 * ep + 2, kcol:kcol + P],
                        ut_s[s][:, 2 * ep:2 * ep + 2, vs:],
                        start=(ep == 0), stop=(ep == DC // 2 - 1),
                        perf_mode=mybir.MatmulPerfMode.DoubleRow)
            else:
                for ec in range(DC):
                    nc.tensor.matmul(
                        acc[:, vs:], xt[:, ec, kcol:kcol + P],
                        ut_s[s][:, ec, vs:],
                        start=(ec == 0), stop=(ec == DC - 1))
            sm_t = p_sm.tile([P, QSUP], F16, tag="sm")
            nc.vector.tensor_add(sm_t[:, vs:], acc[:, vs:],
                                 m_all[:, KT_BASE[s] + kt, vs:])
            nc.scalar.activation(e_ts[s][:, kt, vs:], sm_t[:, vs:],
                                 mybir.ActivationFunctionType.Exp,
                                 scale=SCALE)

        # ---- denominator: d = sum_k E via N=2 matmuls ----
        d_all = p_misc.tile([P, 8], F32, tag="dall")
        for s in range(NSUP):
            e_t = e_ts[s]
            for qs in range(4):
                nav = AVKT[s][qs]
                d_acc = ps512.tile([P, 2], F32, tag="ps512", name="d_acc")
                for kt in range(nav):
                    nc.tensor.matmul(d_acc[:],
                                     e_t[:, kt, qs * P:(qs + 1) * P],
                                     ones_t[:, 0:2],
                                     start=(kt == 0), stop=(kt == nav - 1))
                g = s * 4 + qs
                nc.vector.tensor_copy(d_all[:, g:g + 1], d_acc[:, 0:1])
        nc.sync.dma_start(dsum_d.ap(), d_all[:])

        # ---- zT = x_tok^T @ E per super, then out = zT^T @ Wv ----
        # zT groups are per (super, dc-pair): [P, 2, 512] PSUM (one bank
        # per dc), 512-wide vs-trimmed moving E — same cycles as the AV
        # matmul but the fewest Tensor instructions (the 120KB Tensor
        # instruction stream is demand-paged in 16KB DMAs; its last page
        # arriving late gates the teardown).
        # zT(s1) is emitted before out-proj(s0) so its SBUF copies hide
        # under out-proj(s0)'s matmuls instead of stalling the tail.
        ztt_s = []
        for s in range(NSUP):
            e_t = e_ts[s]
            # uin tiles are dead after u-proj: reuse their SBUF (WAR by tag)
            ztt = p_mw.tile([P, DC, QSUP], F16, tag=f"uin{s}", name=f"ztt{s}")
            ztt_s.append(ztt)
            for dp in range(4):
                zt = pszt.tile([P, 2, QSUP], F32, tag="zt")
                for kt in range(SLOT_KT[s]):
                    vs = _vstart(s, kt)
                    for dc in range(2):
                        dcg = dp * 2 + dc
                        nc.tensor.matmul(
                            zt[:, dc, vs:],
                            xtok_t[:, kt, dcg * P:(dcg + 1) * P],
                            e_t[:, kt, vs:],
                            start=(kt == 0), stop=(kt == SLOT_KT[s] - 1))
                nc.vector.tensor_copy(
                    ztt[:, dp * 2:(dp + 1) * 2, :], zt[:])
        for s in range(NSUP):
            for qp in range(2):
                o_t = p_out.tile([P, 2, D], F16, tag=f"o{s}{qp}",
                                 name=f"o{s}{qp}")
                for qh in range(2):
                    qs = qp * 2 + qh
                    for eh in range(2):
                        acc = ps512.tile([P, QSUP], F32, tag="ps512")
                        for dcg in range(DC):
                            nc.tensor.matmul(
                                acc[:], ztt_s[s][:, dcg, qs * P:(qs + 1) * P],
                                wv_t[:, dcg, eh * QSUP:(eh + 1) * QSUP],
                                start=(dcg == 0), stop=(dcg == DC - 1))
                        nc.scalar.copy(
                            o_t[:, qh, eh * QSUP:(eh + 1) * QSUP], acc[:])
                if s == 1 and qp == 1:
                    # final store: split by partition halves across two
                    # queues to halve the ~4.3us descriptor-service floor
                    nc.sync.dma_start(
                        out_d.ap()[s][0:64, qp * 2:qp * 2 + 2], o_t[0:64])
                    nc.gpsimd.dma_start(
                        out_d.ap()[s][64:P, qp * 2:qp * 2 + 2], o_t[64:P])
                else:
                    nc.sync.dma_start(
                        out_d.ap()[s][:, qp * 2:qp * 2 + 2], o_t[:])

    nc.compile()
    return nc


def _prep_weights(Wq32, Wk32, Wv16):
    """Host-side weight prep (shared by all cores)."""
    M16 = (Wq32 @ Wk32.T).astype(np.float16)
    mw = np.ascontiguousarray(
        M16.reshape(DC, P, DC, P).transpose(1, 2, 0, 3))    # [p, ec, dc, e]
    mw = mw.reshape(P, 2, 4096)                              # per mw-half
    wv = np.ascontiguousarray(Wv16.reshape(DC, P, D).swapaxes(0, 1))
    return mw, wv


def _core_tq(h):
    """Queries for core parity h: every 128-block is split between the
    pair (rows [h*64, h*64+64)), so both cores see identical causal depth
    profiles. Super 0 = blocks 8..15 (ascending depth), super 1 = 0..7."""
    return np.concatenate(
        [bl * P + h * 64 + np.arange(64) for bl in range(8, 16)]
        + [bl * P + h * 64 + np.arange(64) for bl in range(0, 8)])


def _prep_core_inputs(xT16, xtok16, mw, wv, b, h):
    """Host-side shard prep for core (batch b, half h)."""
    tq = _core_tq(h)

    xTb = xT16[b]                                          # [D, T] fp16
    xq = xTb[:, tq].reshape(DC, P, NSUP, QSUP).transpose(2, 1, 0, 3)
    xq = xq.reshape(NSUP, P, 4096)
    uin = [np.ascontiguousarray(
        np.concatenate([mw[:, i], xq[i]], axis=1)) for i in range(2)]
    xt = np.ascontiguousarray(
        xTb.reshape(DC, P, 2, T // 2).transpose(2, 1, 0, 3))
    if SCORES_FP8:
        import ml_dtypes
        xt = xt.astype(ml_dtypes.float8_e4m3)

    masks = np.empty((NKT, P, QSUP), dtype=np.float16)
    base = 0
    for s in range(NSUP):
        kidx = np.arange(SLOT_KT[s] * P).reshape(SLOT_KT[s], P, 1)
        tqs = tq[s * QSUP:(s + 1) * QSUP].reshape(1, 1, QSUP)
        masks[base:base + SLOT_KT[s]] = np.where(
            kidx <= tqs, 0.0, MASK_NEG).astype(np.float16)
        base += SLOT_KT[s]
    import ml_dtypes
    masks = np.ascontiguousarray(masks.transpose(1, 0, 2)).astype(
        ml_dtypes.float8_e5m2)                              # [P, NKT, QSUP]

    im = {"uin0": uin[0], "uin1": uin[1], "wv": wv,
          "msk": masks, "xtok": xtok16[b]}
    for c in range(2):
        im[f"xt{c}"] = xt[c]
    return im, tq


def kernel(x, Wq, Wk, Wv):
    global last_exec_time_ns
    x = np.asarray(x, dtype=np.float32)
    assert x.shape == (B, T, D)

    if "nc" not in _CACHE:
        _CACHE["nc"] = _build_program()
    nc = _CACHE["nc"]

    x16 = x.astype(np.float16)
    xT16 = np.ascontiguousarray(x16.transpose(0, 2, 1))    # [B, D, T]
    xtok16 = np.ascontiguousarray(
        x16.reshape(B, TCH, P, D).transpose(0, 2, 1, 3))   # [B, P, TCH, D]
    mw, wv = _prep_weights(
        np.asarray(Wq, dtype=np.float32),
        np.asarray(Wk, dtype=np.float32),
        np.asarray(Wv, dtype=np.float16))

    in_maps = []
    row_maps = []
    for c in range(8):
        im, tq = _prep_core_inputs(xT16, xtok16, mw, wv, c // 2, c % 2)
        in_maps.append(im)
        row_maps.append(tq)

    trace = bool(os.environ.get("BASS_KERNEL_TRACE"))
    kw = {}
    if trace:
        kw = {"trace": True, "tmpdir": os.environ.get(
            "BASS_KERNEL_TRACE_DIR", "/tmp/kernel_trace")}
    res = run_bass_kernel_spmd(nc, in_maps, core_ids=list(range(8)), **kw)
    if trace:
        last_exec_time_ns = res.exec_time_ns

    out = np.empty((B, T, D), dtype=np.float32)
    for c in range(8):
        o = np.asarray(res.results[c]["out"], dtype=np.float32)  # [2, P, 4, D]
        o = o.transpose(0, 2, 1, 3).reshape(NQ, D)
        d = np.asarray(res.results[c]["dsum"], dtype=np.float32)  # [P, 8]
        o /= np.ascontiguousarray(d.T).reshape(NQ, 1)
        out[c // 2, row_maps[c]] = o
    return out
